# revision 4
# baseline (speedup 1.0000x reference)
"""ConvAConnect TRN2 kernel: per-sample noisy-weight 3x3 conv, data-parallel over 8 cores.

Z[b] = conv2d_valid(X[b], W * Werr[loc_id[b]]) + bias * Berr[loc_id[b]]

Shapes: X[32,64,64,64] f32, W[3,3,64,128], bias[128], Werr[1000,3,3,64,128],
Berr[1000,128], loc_id[32] i32 -> Z[32,62,62,128] f32.

Strategy: shard batch (4 samples/core). Host prep = layout only (X transpose
to cin-major, gather of the 32 needed Werr/Berr pool rows, weight reshapes).
All FLOPs (memW = W*Werr, conv, bias) run on device.

Device kernel per sample:
  - Two stacked SBUF tiles of X^T (cin x H*W grid): XTs1 = [X^T; X^T shifted
    1 pixel], XTs64 = [X^T; X^T shifted 64 pixels]. The 9 conv taps collapse
    to 5 matmuls per 512-pixel output chunk: 4 with K=128 (tap pairs) and one
    K=64 single, accumulated in PSUM. fp32r dtype: full fp32 bits in memory,
    relaxed PE multiply (~2e-4 rel err), 2 cyc/row at N=512.
  - Output grid is 62 rows x 64 cols (2 junk columns keep width-64 alignment
    so every tap is a constant offset); junk columns are dropped at DMA time.
  - ScalarE copies PSUM->SBUF fused with the per-sample bias add; TensorE
    transposes [cout, spatial] -> [spatial, cout] in 128x128 blocks; the
    per-sample result is shipped with two 3D-AP DMAs (even/odd output rows),
    one on each HWDGE ring (sync + scalar).
"""

import sys
import numpy as np

for _p in ("/opt/trn_rl_repo", "/root/.axon_site"):
    if _p not in sys.path:
        sys.path.insert(0, _p)

N_CORES = 8
B = 32
PER_CORE = B // N_CORES
H = Wd = 64
CIN = 64
COUT = 128
HO = WO = 62
GRID = HO * 64          # 62 rows x 64 cols (2 junk cols/row)
XTL = 4104              # X^T free length: 4096 valid + pad (max read 4098)
NCHUNK = 512            # output-grid pixels per PSUM chunk (8 grid rows)
NCHUNKS = 8             # 7 full chunks + 1 of 384

_compiled = {}


def _build():
    import concourse.bass as bass
    import concourse.mybir as mybir
    import concourse.tile as tile
    from concourse import bacc
    from concourse.masks import make_identity

    f32 = mybir.dt.float32
    f32r = mybir.dt.float32r

    nc = bacc.Bacc("TRN2", target_bir_lowering=False, debug=False)

    xt_in = nc.dram_tensor("xt", [PER_CORE, CIN, XTL], f32r, kind="ExternalInput")
    wp_in = nc.dram_tensor("wp", [128, 3 * COUT], f32r, kind="ExternalInput")
    wq_in = nc.dram_tensor("wq", [128, COUT], f32r, kind="ExternalInput")
    ws_in = nc.dram_tensor("ws", [64, COUT], f32r, kind="ExternalInput")
    gp_in = nc.dram_tensor("gp", [PER_CORE, 128, 3 * COUT], f32r, kind="ExternalInput")
    gq_in = nc.dram_tensor("gq", [PER_CORE, 128, COUT], f32r, kind="ExternalInput")
    gs_in = nc.dram_tensor("gs", [PER_CORE, 64, COUT], f32r, kind="ExternalInput")
    bias_in = nc.dram_tensor("bias", [COUT, 1], f32, kind="ExternalInput")
    berr_in = nc.dram_tensor("berr", [COUT, PER_CORE], f32, kind="ExternalInput")
    z_out = nc.dram_tensor("z", [PER_CORE, HO, WO, COUT], f32, kind="ExternalOutput")

    with tile.TileContext(nc) as tc:
        with (
            tc.tile_pool(name="const", bufs=1) as const,
            tc.tile_pool(name="xpool", bufs=2) as xpool,
            tc.tile_pool(name="wpool", bufs=2) as wpool,
            tc.tile_pool(name="spool", bufs=4) as spool,
            tc.tile_pool(name="zpool", bufs=2) as zpool,
            tc.tile_pool(name="psmm", bufs=3, space="PSUM") as psmm,
            tc.tile_pool(name="pst", bufs=4, space="PSUM") as pst,
        ):
            identity = const.tile([128, 128], f32, tag="identity")
            make_identity(nc, identity[:])

            wp_t = const.tile([128, 3 * COUT], f32r, tag="wp")
            wq_t = const.tile([128, COUT], f32r, tag="wq")
            ws_t = const.tile([64, COUT], f32r, tag="ws")
            bias_t = const.tile([COUT, 1], f32, tag="bias")
            berr_t = const.tile([COUT, PER_CORE], f32, tag="berr")
            mb_all = const.tile([COUT, PER_CORE], f32, tag="mb")
            nc.sync.dma_start(wp_t[:], wp_in[:])
            nc.sync.dma_start(wq_t[:], wq_in[:])
            nc.sync.dma_start(ws_t[:], ws_in[:])
            nc.sync.dma_start(bias_t[:], bias_in[:])
            nc.sync.dma_start(berr_t[:], berr_in[:])
            nc.vector.tensor_scalar_mul(mb_all[:], berr_t[:], bias_t[:])

            for b in range(PER_CORE):
                xts1 = xpool.tile([128, XTL], f32r, tag="xts1")
                nc.sync.dma_start(xts1[0:64, :], xt_in[b])
                nc.sync.dma_start(xts1[64:128, 0 : XTL - 1], xt_in[b][:, 1:XTL])
                xts64 = xpool.tile([128, XTL], f32r, tag="xts64")
                nc.scalar.dma_start(xts64[0:64, :], xt_in[b])
                nc.scalar.dma_start(xts64[64:128, 0 : XTL - 64], xt_in[b][:, 64:XTL])

                gpt = wpool.tile([128, 3 * COUT], f32r, tag="gpt")
                gqt = wpool.tile([128, COUT], f32r, tag="gqt")
                gst = wpool.tile([64, COUT], f32r, tag="gst")
                nc.sync.dma_start(gpt[:], gp_in[b])
                nc.sync.dma_start(gqt[:], gq_in[b])
                nc.sync.dma_start(gst[:], gs_in[b])
                mwp = wpool.tile([128, 3 * COUT], f32r, tag="mwp")
                mwq = wpool.tile([128, COUT], f32r, tag="mwq")
                mws = wpool.tile([64, COUT], f32r, tag="mws")
                nc.vector.tensor_mul(mwp[:], wp_t[:], gpt[:])
                nc.vector.tensor_mul(mwq[:], wq_t[:], gqt[:])
                nc.vector.tensor_mul(mws[:], ws_t[:], gst[:])

                zbuf = zpool.tile([128, 31 * 128], f32, tag="zbuf")

                for c in range(NCHUNKS):
                    base = c * NCHUNK
                    ncols = min(NCHUNK, GRID - base)
                    pc = psmm.tile([128, NCHUNK], f32, tag="pc")
                    # taps (fh,0)+(fh,1): K=128 pairs from the shift-1 stack
                    for fh in range(3):
                        nc.tensor.matmul(
                            pc[:, :ncols],
                            mwp[:, fh * COUT : (fh + 1) * COUT],
                            xts1[:, base + fh * 64 : base + fh * 64 + ncols],
                            start=(fh == 0),
                            stop=False,
                        )
                    # taps (0,2)+(1,2): K=128 pair from the shift-64 stack
                    nc.tensor.matmul(
                        pc[:, :ncols],
                        mwq[:],
                        xts64[:, base + 2 : base + 2 + ncols],
                        start=False,
                        stop=False,
                    )
                    # tap (2,2): K=64 single (top half of xts1 is unshifted X^T)
                    nc.tensor.matmul(
                        pc[:, :ncols],
                        mws[:],
                        xts1[0:64, base + 130 : base + 130 + ncols],
                        start=False,
                        stop=True,
                    )
                    out_s = spool.tile([128, NCHUNK], f32, tag="outs")
                    nc.scalar.activation(
                        out_s[:, :ncols],
                        pc[:, :ncols],
                        mybir.ActivationFunctionType.Identity,
                        bias=mb_all[:, b : b + 1],
                    )
                    for k in range(ncols // 128):
                        pt = pst.tile([128, 128], f32, tag="pt")
                        nc.tensor.transpose(
                            pt[:], out_s[:, k * 128 : (k + 1) * 128], identity[:]
                        )
                        j = c * 4 + k  # grid row-pair index, 0..30
                        nc.vector.tensor_copy(
                            zbuf[:, j * 128 : (j + 1) * 128], pt[:]
                        )

                # ship the sample: even rows on the sync ring, odd on scalar
                dst = z_out[b].rearrange("(j a) p o -> a p j o", a=2)
                nc.sync.dma_start(
                    dst[0], zbuf[0:62, :].rearrange("p (j o) -> p j o", j=31)
                )
                nc.scalar.dma_start(
                    dst[1], zbuf[64:126, :].rearrange("p (j o) -> p j o", j=31)
                )

    nc.compile()
    return nc


def _get_nc():
    if "nc" not in _compiled:
        _compiled["nc"] = _build()
    return _compiled["nc"]


def _prep_inputs(X, W, bias, Werr, Berr, loc_id):
    """Host-side shard/layout prep. Returns per-core in_maps."""
    X = np.asarray(X, dtype=np.float32)
    W = np.asarray(W, dtype=np.float32)
    bias = np.asarray(bias, dtype=np.float32)
    Werr = np.asarray(Werr, dtype=np.float32)
    Berr = np.asarray(Berr, dtype=np.float32)
    loc_id = np.asarray(loc_id)

    # X^T: [B, CIN, H*W] padded to XTL
    xt = np.zeros((B, CIN, XTL), dtype=np.float32)
    xt[:, :, : H * Wd] = X.transpose(0, 3, 1, 2).reshape(B, CIN, H * Wd)

    # wp[fw*64+cin, fh*128+cout] = W[fh, fw, cin, cout] for fw in {0,1}
    wp = np.ascontiguousarray(W[:, :2].transpose(1, 2, 0, 3).reshape(128, 3 * COUT))
    # wq[fh*64+cin, cout] = W[fh, 2, cin, cout] for fh in {0,1}
    wq = np.ascontiguousarray(W[:2, 2].reshape(128, COUT))
    # ws[cin, cout] = W[2, 2, cin, cout]
    ws = np.ascontiguousarray(W[2, 2])

    g = Werr[loc_id]  # [B, 3, 3, 64, 128]
    gp = np.ascontiguousarray(
        g[:, :, :2].transpose(0, 2, 3, 1, 4).reshape(B, 128, 3 * COUT)
    )
    gq = np.ascontiguousarray(g[:, :2, 2].reshape(B, 128, COUT))
    gs = np.ascontiguousarray(g[:, 2, 2])

    be = Berr[loc_id]  # [B, 128]
    bias_col = np.ascontiguousarray(bias.reshape(COUT, 1))

    in_maps = []
    for i in range(N_CORES):
        s = slice(i * PER_CORE, (i + 1) * PER_CORE)
        in_maps.append(
            {
                "xt": np.ascontiguousarray(xt[s]),
                "wp": wp,
                "wq": wq,
                "ws": ws,
                "gp": np.ascontiguousarray(gp[s]),
                "gq": np.ascontiguousarray(gq[s]),
                "gs": np.ascontiguousarray(gs[s]),
                "bias": bias_col,
                "berr": np.ascontiguousarray(be[s].T),
            }
        )
    return in_maps


def _run(in_maps, trace=False, **kw):
    from concourse.bass_utils import run_bass_kernel_spmd

    nc = _get_nc()
    return run_bass_kernel_spmd(nc, in_maps, list(range(N_CORES)), trace=trace, **kw)


def kernel(X, W, bias, Werr, Berr, loc_id):
    in_maps = _prep_inputs(X, W, bias, Werr, Berr, loc_id)
    res = _run(in_maps)
    return np.concatenate([res.results[i]["z"] for i in range(N_CORES)], axis=0)
